# revision 33
# baseline (speedup 1.0000x reference)
"""Multi-head causal self-attention on 8 TRN2 NeuronCores.

Problem (nn_MultiHeadAttention): B=2, T=2048, C=1024, H=16 heads, hs=64.
  q,k,v = per-head projections of x; causal softmax(q k^T / 8) v;
  concat heads; out = att @ Wo + bo.

Sharding: core c in 0..7 -> (batch b = c//4, head-group g = c%4, 4 heads each).
Each core computes Q/K/V + flash-style causal attention for its 4 heads on its
batch, normalized attention outputs are AllGathered across the 4 cores of the
same batch (replica groups [0-3], [4-7]), then each core computes a disjoint
256-column slice of the output projection (column-parallel Wo) + bias slice.
Host does a pure concat of the 8 disjoint output slices.

All matmuls run as float32r (single-pass fp32 PE mode, 4x faster than fp32).
Attention works in transposed layout throughout: Q^T/K^T [d, t], scores
S^T [s, t], P^T = exp(S^T/8) with causal mask, att^T [d, t] via
lhsT=[V_h | ones] (row 64 of the PSUM accumulator = softmax denominator).
Normalization multiplies by a PE-broadcast reciprocal row.

Scheduling notes: per-engine instruction order is static, so projection
(stage 1) and output-projection (stage 3) work is interleaved into the
attention head loops to fill PE bubbles left by the scores->exp->AV chain,
and stage-3 matmuls for t-block qb are emitted only during stage-2 of qb+1,
when their AllGathered inputs have already landed.
"""

import numpy as np
from contextlib import ExitStack

import concourse.bass as bass
import concourse.mybir as mybir
import concourse.tile as tile
from concourse import bacc
from concourse.bass_utils import run_bass_kernel_spmd

F32 = mybir.dt.float32
F32R = mybir.dt.float32r
EXP = mybir.ActivationFunctionType.Exp

N_CORES = 8
B = 2
T = 2048
C = 1024
NH = 16
HS = 64
E = 1024
GROUPS = 4          # head groups (tensor-parallel ranks per batch)
HPG = NH // GROUPS  # 4 heads per core
ES = E // GROUPS    # 256 output columns per core
HD = HPG * HS       # 256 local attention-output rows

P = 128             # partition tile
TBLK = 512          # t-block (matmul moving dim)
NTB = T // TBLK     # 4
NCT = C // P        # 8 contraction tiles for projections
NST = T // P        # 16 key tiles
VW = HS + 1         # V lhsT width per head (64 V cols + ones col)

REPLICA_GROUPS = [[0, 1, 2, 3], [4, 5, 6, 7]]


def build_nc(with_collective=True):
    """Build + compile the per-core SPMD program. Same program on all cores."""
    nc = bacc.Bacc(
        "TRN2", target_bir_lowering=False, debug=False, num_devices=N_CORES
    )

    xT = nc.dram_tensor("xT", [C, T], F32R, kind="ExternalInput").ap()
    wq = nc.dram_tensor("wq", [C, HD], F32R, kind="ExternalInput").ap()
    wk = nc.dram_tensor("wk", [C, HD], F32R, kind="ExternalInput").ap()
    wv = nc.dram_tensor("wv", [C, HD], F32R, kind="ExternalInput").ap()
    wo = nc.dram_tensor("wo", [E, ES], F32R, kind="ExternalInput").ap()
    bo = nc.dram_tensor("bo", [1, ES], F32R, kind="ExternalInput").ap()
    tri = nc.dram_tensor("tri", [P, P], F32R, kind="ExternalInput").ap()
    onesc = nc.dram_tensor("onesc", [1, P], F32R, kind="ExternalInput").ap()
    vones = nc.dram_tensor("vones", [P, HPG], F32R, kind="ExternalInput").ap()
    out = nc.dram_tensor("out", [T, ES], F32, kind="ExternalOutput").ap()

    with tile.TileContext(nc) as tc, ExitStack() as ctx:
        wp = ctx.enter_context(tc.tile_pool(name="wp", bufs=1))
        xp = ctx.enter_context(tc.tile_pool(name="xp", bufs=2))
        qkp = ctx.enter_context(tc.tile_pool(name="qkp", bufs=1))
        vp = ctx.enter_context(tc.tile_pool(name="vp", bufs=1))
        ptp = ctx.enter_context(tc.tile_pool(name="ptp", bufs=6))
        attp = ctx.enter_context(tc.tile_pool(name="attp", bufs=2))
        smp = ctx.enter_context(tc.tile_pool(name="smp", bufs=3))
        outp = ctx.enter_context(tc.tile_pool(name="outp", bufs=3))
        lhp = ctx.enter_context(tc.tile_pool(name="lhp", bufs=9))
        # PSUM: 8 banks total.  st2 [128,1024] = 2 banks x 2 bufs = 4,
        # attv 1 bank x 2, small (bc / out-proj) 1 bank x 2.
        ps2 = ctx.enter_context(tc.tile_pool(name="ps2", bufs=2, space="PSUM"))
        psB = ctx.enter_context(tc.tile_pool(name="psB", bufs=2, space="PSUM"))
        psC = ctx.enter_context(tc.tile_pool(name="psC", bufs=2, space="PSUM"))
        dramp = ctx.enter_context(tc.tile_pool(name="dramp", bufs=1, space="DRAM"))

        # ---- small constants ----
        ones = wp.tile([1, P], F32R, tag="ones")
        nc.sync.dma_start(ones[:], onesc[:])
        tri_sb = wp.tile([P, P], F32R, tag="tri")
        nc.sync.dma_start(tri_sb[:], tri[:])
        bias_sb = wp.tile([1, ES], F32R, tag="bias")

        w_sb = {n: [] for n in ("wq", "wk", "wv", "wo")}
        for name in ("wq", "wk", "wv", "wo"):
            for ci in range(NCT):
                w_sb[name].append(
                    wp.tile([P, ES], F32R, tag=f"{name}{ci}", name=f"{name}{ci}")
                )

        # x^T tiles per (c-tile, t-block), double-buffered across t-blocks:
        # x(tb) is only read by stage-1(tb), so two t-blocks' worth suffices
        xt_of = {}

        def alloc_xt(tb):
            xt_of[tb] = [
                xp.tile([P, TBLK], F32R, tag=f"x{ci}", name=f"x{ci}_{tb}")
                for ci in range(NCT)
            ]
            return xt_of[tb]

        # merged Q^T/K^T per head pair: col = tb*1024 + qk*512 + t_local
        # (pair p holds heads 2p (rows 0-63) and 2p+1 (rows 64-127))
        qkt = [qkp.tile([P, 2 * T], F32R, tag=f"qk{p_}", name=f"qk{p_}")
               for p_ in range(2)]

        def qt_slice(pr, r0, rn, t0, tn):
            tb, tl = t0 // TBLK, t0 % TBLK
            base = tb * 1024 + tl
            return qkt[pr][r0:r0 + rn, base:base + tn]

        def kt_slice(pr, r0, rn, s0, sn):
            tb, sl = s0 // TBLK, s0 % TBLK
            base = tb * 1024 + TBLK + sl
            return qkt[pr][r0:r0 + rn, base:base + sn]

        v_sb = [vp.tile([P, HPG * VW], F32R, tag=f"v{st}", name=f"v{st}")
                for st in range(NST)]

        # ---------------- stage-1 pieces ----------------
        def emit_x_dma(tb):
            ts_ = tb * TBLK
            xt = alloc_xt(tb)
            for ci in range(NCT):
                nc.sync.dma_start(
                    xt[ci][:], xT[ci * P:(ci + 1) * P, ts_:ts_ + TBLK])

        def emit_qk_proj(tb, pr):
            xt = xt_of[tb]
            ps = ps2.tile([P, 2 * TBLK], F32, tag="st2", name=f"qkps{tb}_{pr}")
            for ci in range(NCT):
                nc.tensor.matmul(
                    ps[:, 0:TBLK],
                    lhsT=w_sb["wq"][ci][:, pr * P:(pr + 1) * P],
                    rhs=xt[ci][:],
                    start=(ci == 0), stop=(ci == NCT - 1),
                )
            for ci in range(NCT):
                nc.tensor.matmul(
                    ps[:, TBLK:2 * TBLK],
                    lhsT=w_sb["wk"][ci][:, pr * P:(pr + 1) * P],
                    rhs=xt[ci][:],
                    start=(ci == 0), stop=(ci == NCT - 1),
                )
            nc.vector.tensor_copy(qkt[pr][:, tb * 1024:(tb + 1) * 1024], ps[:])

        def emit_v_proj(st):
            tb, sl = st // 4, (st % 4) * P
            xt = xt_of[tb]
            vps = ps2.tile([P, 2 * TBLK], F32, tag="st2", name=f"vps{st}")
            for ci in range(NCT):
                nc.tensor.matmul(
                    vps[:, 0:HD],
                    lhsT=xt[ci][:, sl:sl + P],
                    rhs=w_sb["wv"][ci][:],
                    start=(ci == 0), stop=(ci == NCT - 1),
                )
            nc.sync.dma_start(
                v_sb[st][:].rearrange("p (h x) -> p h x", h=HPG)[:, :, HS:VW],
                vones[:].rearrange("p (h o) -> p h o", o=1),
            )
            nc.vector.tensor_copy(
                v_sb[st][:].rearrange("p (h x) -> p h x", h=HPG)[:, :, 0:HS],
                vps[:, 0:HD].rearrange("p (h x) -> p h x", h=HPG),
            )

        def qk_chunks(tb):
            return [lambda tb=tb: emit_qk_proj(tb, 0),
                    lambda tb=tb: emit_qk_proj(tb, 1)]

        def v_chunks(tb):
            return [lambda st=st: emit_v_proj(st)
                    for st in range(4 * tb, 4 * tb + 4)]

        # ------- stage-2 piece (one head PAIR of one t-block, jointly) ------
        def emit_headpair(qb, pr, attn_pair):
            """Process both heads of qkt pair `pr` together: the two score
            matmuls for one s-tile live in disjoint PE row-groups (lhsT rows
            0-63 vs 64-127) and run concurrently on hardware; one [128,1024]
            ACT exp covers both heads.  Yields once per s-tile so the driver
            can weave filler PE work into the exp-latency bubbles."""
            t0 = qb * TBLK
            ns = 4 * (qb + 1)
            attv = [
                psB.tile([VW, TBLK], F32, tag="attv", name=f"attv{qb}_{pr}_{par}")
                for par in range(2)
            ]
            for si in range(ns):
                diag = si * P >= t0
                ka = si * P - t0 if diag else 0
                stp = ps2.tile([P, 2 * TBLK], F32, tag="st2",
                               name=f"st{qb}_{pr}_{si}")
                for par in range(2):
                    r0 = par * HS
                    nc.tensor.matmul(
                        stp[:, par * TBLK:(par + 1) * TBLK],
                        lhsT=kt_slice(pr, r0, HS, si * P, P),
                        rhs=qt_slice(pr, r0, HS, t0, TBLK),
                        start=True, stop=True,
                    )
                pt = ptp.tile([P, 2 * TBLK], F32R, tag="pt",
                              name=f"pt{qb}_{pr}_{si}")
                if diag:
                    for par in range(2):
                        c0 = par * TBLK + ka
                        nc.scalar.activation(
                            pt[:, c0:(par + 1) * TBLK],
                            stp[:, c0:(par + 1) * TBLK], EXP, scale=0.125)
                        nc.vector.tensor_mul(
                            pt[:, c0:c0 + P], pt[:, c0:c0 + P], tri_sb[:])
                else:
                    nc.scalar.activation(pt[:], stp[:], EXP, scale=0.125)
                for par in range(2):
                    h = 2 * pr + par
                    nc.tensor.matmul(
                        attv[par][:, ka:TBLK],
                        lhsT=v_sb[si][:, h * VW:(h + 1) * VW],
                        rhs=pt[:, par * TBLK + ka:(par + 1) * TBLK],
                        start=(si == 0), stop=(si == ns - 1),
                    )
                yield
            # normalize: recip of denominator row, PE-broadcast, multiply
            for par in range(2):
                r0 = par * HS
                recip = smp.tile([1, TBLK], F32R, tag="recip")
                with nc.allow_low_precision(
                    reason="f32r reciprocal feeds PE broadcast; 19-bit "
                    "mantissa is ample for softmax denominators"
                ):
                    nc.vector.reciprocal(recip[:], attv[par][HS:HS + 1, :])
                bc = psC.tile([HS, TBLK], F32, tag="small",
                              name=f"bc{qb}_{pr}_{par}")
                nc.tensor.matmul(
                    bc[:], lhsT=ones[0:1, 0:HS], rhs=recip[:],
                    start=True, stop=True,
                )
                bcs = smp.tile([HS, TBLK], F32, tag="bcs")
                nc.vector.tensor_copy(bcs[:], bc[:])
                nc.vector.tensor_mul(
                    attn_pair[pr][r0:r0 + HS, :], attv[par][0:HS, :], bcs[:]
                )

        # ---------------- stage-3 piece (one t-tile of one t-block) ---------
        def emit_oproj_tt(qb, lh, tt):
            # lh[hdt] holds att^T rows for global heads (2*hdt, 2*hdt+1)...
            # here indexed so lh[hdt] pairs with w_sb["wo"][hdt]
            t0 = qb * TBLK
            op = psC.tile([P, ES], F32, tag="small", name=f"op{qb}_{tt}")
            nc.tensor.matmul(
                op[:], lhsT=ones[0:1, :], rhs=bias_sb[:],
                start=True, stop=False,
            )
            # pr0 tiles (even hdt) first: they arrive one AllGather earlier
            order = [0, 2, 4, 6, 1, 3, 5, 7]
            for i, hdt in enumerate(order):
                nc.tensor.matmul(
                    op[:],
                    lhsT=lh[hdt][:, tt * P:(tt + 1) * P],
                    rhs=w_sb["wo"][hdt][:],
                    start=False,
                    stop=(i == NCT - 1),
                )
            osb = outp.tile([P, ES], F32, tag="osb", name=f"osb{qb}_{tt}")
            nc.scalar.copy(osb[:], op[:])
            nc.sync.dma_start(out[t0 + tt * P:t0 + (tt + 1) * P, :], osb[:])

        # --------- per-pair AllGather (pr = head pair 0/1 of this core) -----
        # Gathering one head-pair [128, 512] per collective: output rows are
        # rank-major, i.e. block g holds GLOBAL heads (4g+2pr, 4g+2pr+1) =
        # global hd-tile index 2g+pr.  lh list is indexed by wo-row tile.
        def emit_ag(qb, pr, attn_pair, lh):
            ag_in = dramp.tile([P, TBLK], F32R, tag=f"agin{qb}_{pr}")
            nc.sync.dma_start(ag_in[:], attn_pair[pr][:])
            ag_out = dramp.tile([GROUPS * P, TBLK], F32R, tag=f"agout{qb}_{pr}")
            if with_collective:
                nc.gpsimd.collective_compute(
                    "AllGather",
                    mybir.AluOpType.bypass,
                    replica_groups=REPLICA_GROUPS,
                    ins=[ag_in[:].opt()],
                    outs=[ag_out[:].opt()],
                )
            else:  # timing/sim variant: fake the AG with local DMA copies
                for g_ in range(GROUPS):
                    nc.sync.dma_start(
                        ag_out[g_ * P:(g_ + 1) * P, :], ag_in[:])
            for g_ in range(GROUPS):
                t_ = lhp.tile([P, TBLK], F32R, tag="lh",
                              name=f"lh{qb}_{pr}_{g_}")
                nc.sync.dma_start(t_[:], ag_out[g_ * P:(g_ + 1) * P, :])
                lh[2 * g_ + pr] = t_

        # ---------------- emission schedule ----------------
        # stage 1, t-block 0 (DMAs interleaved for fast start)
        xt0 = alloc_xt(0)
        for ci in range(NCT):
            nc.sync.dma_start(w_sb["wq"][ci][:], wq[ci * P:(ci + 1) * P, :])
            nc.sync.dma_start(xt0[ci][:], xT[ci * P:(ci + 1) * P, 0:TBLK])
        for name, src in (("wk", wk), ("wv", wv)):
            for ci in range(NCT):
                nc.sync.dma_start(w_sb[name][ci][:], src[ci * P:(ci + 1) * P, :])
        for chunk in qk_chunks(0) + v_chunks(0):
            chunk()

        lh_of = {}
        for qb in range(NTB):
            if qb + 1 < NTB:
                emit_x_dma(qb + 1)
            if qb == 0:
                # wo/bias DMAs: needed only from stage 3 on, so they queue
                # behind the t-block-1 x loads
                for ci in range(NCT):
                    nc.sync.dma_start(
                        w_sb["wo"][ci][:], wo[ci * P:(ci + 1) * P, :])
                nc.sync.dma_start(bias_sb[:], bo[:])
            # V projections of t-block qb (deferred from stage 1): consumed
            # by this block's own diagonal s-tiles, so they are injected at
            # units 2/4/6/8 of the first head pair.  Other fillers (next
            # block's Q/K projections, previous block's out-projection) are
            # spread evenly.
            vfill = v_chunks(qb) if qb > 0 else []
            fillers = qk_chunks(qb + 1) if qb + 1 < NTB else []
            if qb > 0:
                fillers += [
                    (lambda tt=tt, q=qb - 1: emit_oproj_tt(q, lh_of[q], tt))
                    for tt in range(4)
                ]
            total_units = 2 * 4 * (qb + 1)
            stride = max(2, total_units // max(1, len(fillers)))
            attn_pair = [
                attp.tile([P, TBLK], F32R, tag=f"attn{p_}", name=f"at{qb}_{p_}")
                for p_ in range(2)
            ]
            lh = [None] * NCT
            ctr = 0
            for pr in range(2):
                for _ in emit_headpair(qb, pr, attn_pair):
                    ctr += 1
                    if vfill and ctr % 2 == 1:
                        vfill.pop(0)()
                    elif fillers and ctr % stride == 0:
                        fillers.pop(0)()
                if pr == 0:
                    emit_ag(qb, 0, attn_pair, lh)
            while vfill:
                vfill.pop(0)()
            while fillers:
                fillers.pop(0)()
            emit_ag(qb, 1, attn_pair, lh)
            lh_of[qb] = lh

        # tail: out-projection of the last t-block
        for tt in range(4):
            emit_oproj_tt(NTB - 1, lh_of[NTB - 1], tt)

    nc.compile()
    return nc


_NC_CACHE = {}


def _get_nc(with_collective=True):
    key = with_collective
    if key not in _NC_CACHE:
        _NC_CACHE[key] = build_nc(with_collective)
    return _NC_CACHE[key]


def make_in_maps(x, Wq, Wk, Wv, Wo, bo):
    tri = np.ascontiguousarray(np.triu(np.ones((P, P), dtype=np.float32)))
    onesc = np.ones((1, P), dtype=np.float32)
    vones = np.ones((P, HPG), dtype=np.float32)
    in_maps = []
    for c in range(N_CORES):
        b, g = c // GROUPS, c % GROUPS
        hs_ = slice(g * HPG, (g + 1) * HPG)
        in_maps.append({
            "xT": np.ascontiguousarray(x[b].T),
            "wq": np.ascontiguousarray(
                Wq[hs_].transpose(1, 0, 2).reshape(C, HD)),
            "wk": np.ascontiguousarray(
                Wk[hs_].transpose(1, 0, 2).reshape(C, HD)),
            "wv": np.ascontiguousarray(
                Wv[hs_].transpose(1, 0, 2).reshape(C, HD)),
            "wo": np.ascontiguousarray(Wo[:, g * ES:(g + 1) * ES]),
            "bo": np.ascontiguousarray(bo[g * ES:(g + 1) * ES].reshape(1, ES)),
            "tri": tri,
            "onesc": onesc,
            "vones": vones,
        })
    return in_maps


def kernel(x, Wq, Wk, Wv, Wo, bo):
    x = np.asarray(x, dtype=np.float32)
    Wq = np.asarray(Wq, dtype=np.float32)
    Wk = np.asarray(Wk, dtype=np.float32)
    Wv = np.asarray(Wv, dtype=np.float32)
    Wo = np.asarray(Wo, dtype=np.float32)
    bo = np.asarray(bo, dtype=np.float32)

    nc = _get_nc(with_collective=True)
    in_maps = make_in_maps(x, Wq, Wk, Wv, Wo, bo)
    res = run_bass_kernel_spmd(nc, in_maps, core_ids=list(range(N_CORES)))

    out = np.empty((B, T, E), dtype=np.float32)
    for c in range(N_CORES):
        b, g = c // GROUPS, c % GROUPS
        out[b, :, g * ES:(g + 1) * ES] = res.results[c]["out"]
    return out


# revision 43
# speedup vs baseline: 1.0798x; 1.0798x over previous
"""Multi-head causal self-attention on 8 TRN2 NeuronCores.

Problem (nn_MultiHeadAttention): B=2, T=2048, C=1024, H=16 heads, hs=64.
  q,k,v = per-head projections of x; causal softmax(q k^T / 8) v;
  concat heads; out = att @ Wo + bo.

Sharding: core c in 0..7 -> (batch b = c//4, head-group g = c%4, 4 heads each).
Each core computes Q/K/V + flash-style causal attention for its 4 heads on its
batch, normalized attention outputs are AllGathered across the 4 cores of the
same batch (replica groups [0-3], [4-7]), then each core computes a disjoint
256-column slice of the output projection (column-parallel Wo) + bias slice.
Host does a pure concat of the 8 disjoint output slices.

All matmuls run as float32r (single-pass fp32 PE mode, 4x faster than fp32).
Attention works in transposed layout throughout: Q^T/K^T [d, t], scores
S^T [s, t], P^T = exp(S^T/8) with causal mask, att^T [d, t] via
lhsT=[V_h | ones] (row 64 of the PSUM accumulator = softmax denominator).
Normalization multiplies by a PE-broadcast reciprocal row.

Scheduling notes: per-engine instruction order is static, so projection
(stage 1) and output-projection (stage 3) work is interleaved into the
attention head loops to fill PE bubbles left by the scores->exp->AV chain,
and stage-3 matmuls for t-block qb are emitted only during stage-2 of qb+1,
when their AllGathered inputs have already landed.
"""

import numpy as np
from contextlib import ExitStack

import concourse.bass as bass
import concourse.mybir as mybir
import concourse.tile as tile
from concourse import bacc
from concourse.bass_utils import run_bass_kernel_spmd

F32 = mybir.dt.float32
F32R = mybir.dt.float32r
EXP = mybir.ActivationFunctionType.Exp

N_CORES = 8
B = 2
T = 2048
C = 1024
NH = 16
HS = 64
E = 1024
GROUPS = 4          # head groups (tensor-parallel ranks per batch)
HPG = NH // GROUPS  # 4 heads per core
ES = E // GROUPS    # 256 output columns per core
HD = HPG * HS       # 256 local attention-output rows

P = 128             # partition tile
TBLK = 512          # t-block (matmul moving dim)
NTB = T // TBLK     # 4
NCT = C // P        # 8 contraction tiles for projections
NST = T // P        # 16 key tiles
VW = HS + 1         # V lhsT width per head (64 V cols + ones col)

REPLICA_GROUPS = [[0, 1, 2, 3], [4, 5, 6, 7]]


def build_nc(with_collective=True):
    """Build + compile the per-core SPMD program. Same program on all cores."""
    nc = bacc.Bacc(
        "TRN2", target_bir_lowering=False, debug=False, num_devices=N_CORES
    )

    xT = nc.dram_tensor("xT", [C, T], F32R, kind="ExternalInput").ap()
    wq = nc.dram_tensor("wq", [C, HD], F32R, kind="ExternalInput").ap()
    wk = nc.dram_tensor("wk", [C, HD], F32R, kind="ExternalInput").ap()
    wv = nc.dram_tensor("wv", [C, HD], F32R, kind="ExternalInput").ap()
    wo = nc.dram_tensor("wo", [E, ES], F32R, kind="ExternalInput").ap()
    bo = nc.dram_tensor("bo", [1, ES], F32R, kind="ExternalInput").ap()
    tri = nc.dram_tensor("tri", [P, P], F32R, kind="ExternalInput").ap()
    onesc = nc.dram_tensor("onesc", [1, P], F32R, kind="ExternalInput").ap()
    vones = nc.dram_tensor("vones", [P, HPG], F32R, kind="ExternalInput").ap()
    out = nc.dram_tensor("out", [T, ES], F32, kind="ExternalOutput").ap()

    with tile.TileContext(nc) as tc, ExitStack() as ctx:
        wp = ctx.enter_context(tc.tile_pool(name="wp", bufs=1))
        xp = ctx.enter_context(tc.tile_pool(name="xp", bufs=2))
        qkp = ctx.enter_context(tc.tile_pool(name="qkp", bufs=1))
        vp = ctx.enter_context(tc.tile_pool(name="vp", bufs=1))
        ptp = ctx.enter_context(tc.tile_pool(name="ptp", bufs=6))
        attp = ctx.enter_context(tc.tile_pool(name="attp", bufs=2))
        smp = ctx.enter_context(tc.tile_pool(name="smp", bufs=3))
        outp = ctx.enter_context(tc.tile_pool(name="outp", bufs=3))
        lhp = ctx.enter_context(tc.tile_pool(name="lhp", bufs=9))
        # PSUM: 8 banks total.  st2 [128,1024] = 2 banks x 2 bufs = 4,
        # attv 1 bank x 2, small (bc / out-proj) 1 bank x 2.
        ps2 = ctx.enter_context(tc.tile_pool(name="ps2", bufs=2, space="PSUM"))
        psB = ctx.enter_context(tc.tile_pool(name="psB", bufs=2, space="PSUM"))
        psC = ctx.enter_context(tc.tile_pool(name="psC", bufs=2, space="PSUM"))
        dramp = ctx.enter_context(tc.tile_pool(name="dramp", bufs=1, space="DRAM"))

        # ---- small constants ----
        ones = wp.tile([1, P], F32R, tag="ones")
        nc.sync.dma_start(ones[:], onesc[:])
        tri_sb = wp.tile([P, P], F32R, tag="tri")
        nc.sync.dma_start(tri_sb[:], tri[:])
        bias_sb = wp.tile([1, ES], F32R, tag="bias")

        w_sb = {n: [] for n in ("wq", "wk", "wv", "wo")}
        for name in ("wq", "wk", "wv", "wo"):
            for ci in range(NCT):
                w_sb[name].append(
                    wp.tile([P, ES], F32R, tag=f"{name}{ci}", name=f"{name}{ci}")
                )

        # x^T tiles per (c-tile, t-block), double-buffered across t-blocks:
        # x(tb) is only read by stage-1(tb), so two t-blocks' worth suffices
        xt_of = {}

        def alloc_xt(tb):
            xt_of[tb] = [
                xp.tile([P, TBLK], F32R, tag=f"x{ci}", name=f"x{ci}_{tb}")
                for ci in range(NCT)
            ]
            return xt_of[tb]

        # merged Q^T/K^T per head pair: col = tb*1024 + qk*512 + t_local
        # (pair p holds heads 2p (rows 0-63) and 2p+1 (rows 64-127))
        qkt = [qkp.tile([P, 2 * T], F32R, tag=f"qk{p_}", name=f"qk{p_}")
               for p_ in range(2)]

        def qt_slice(pr, r0, rn, t0, tn):
            tb, tl = t0 // TBLK, t0 % TBLK
            base = tb * 1024 + tl
            return qkt[pr][r0:r0 + rn, base:base + tn]

        def kt_slice(pr, r0, rn, s0, sn):
            tb, sl = s0 // TBLK, s0 % TBLK
            base = tb * 1024 + TBLK + sl
            return qkt[pr][r0:r0 + rn, base:base + sn]

        v_sb = [vp.tile([P, HPG * VW], F32R, tag=f"v{st}", name=f"v{st}")
                for st in range(NST)]

        # ---------------- stage-1 pieces ----------------
        def emit_x_dma(tb):
            # SWDGE queues (gpsimd): runs parallel to the HWDGE weight loads
            ts_ = tb * TBLK
            xt = alloc_xt(tb)
            for ci in range(NCT):
                nc.gpsimd.dma_start(
                    xt[ci][:], xT[ci * P:(ci + 1) * P, ts_:ts_ + TBLK])

        def emit_qk_proj(tb, pr, which):
            # one [128,512] accumulation on the psC "small" tag (see
            # emit_v_proj for why not st2); which=0 -> Q, which=1 -> K
            xt = xt_of[tb]
            wn = "wq" if which == 0 else "wk"
            ps = psC.tile([P, TBLK], F32, tag="small",
                          name=f"qkps{tb}_{pr}_{which}")
            for ci in range(NCT):
                nc.tensor.matmul(
                    ps[:],
                    lhsT=w_sb[wn][ci][:, pr * P:(pr + 1) * P],
                    rhs=xt[ci][:],
                    start=(ci == 0), stop=(ci == NCT - 1),
                )
            base = tb * 1024 + which * TBLK
            nc.vector.tensor_copy(qkt[pr][:, base:base + TBLK], ps[:])

        def emit_v_proj(st):
            # psC "small" tag, NOT ps2: a V filler holding an st2 slot for its
            # 8-matmul group would degrade the scores/exp pipeline to
            # single-buffering
            tb, sl = st // 4, (st % 4) * P
            xt = xt_of[tb]
            vps = psC.tile([P, TBLK], F32, tag="small", name=f"vps{st}")
            for ci in range(NCT):
                nc.tensor.matmul(
                    vps[:, 0:HD],
                    lhsT=xt[ci][:, sl:sl + P],
                    rhs=w_sb["wv"][ci][:],
                    start=(ci == 0), stop=(ci == NCT - 1),
                )
            nc.sync.dma_start(
                v_sb[st][:].rearrange("p (h x) -> p h x", h=HPG)[:, :, HS:VW],
                vones[:].rearrange("p (h o) -> p h o", o=1),
            )
            nc.vector.tensor_copy(
                v_sb[st][:].rearrange("p (h x) -> p h x", h=HPG)[:, :, 0:HS],
                vps[:, 0:HD].rearrange("p (h x) -> p h x", h=HPG),
            )

        def qk_chunks(tb):
            return [lambda tb=tb, pr=pr, w=w: emit_qk_proj(tb, pr, w)
                    for pr in range(2) for w in range(2)]

        def v_chunks(tb):
            return [lambda st=st: emit_v_proj(st)
                    for st in range(4 * tb, 4 * tb + 4)]

        # ------- stage-2 piece (one head PAIR of one t-block, jointly) ------
        def emit_headpair(qb, pr, attn_pair):
            """Process both heads of qkt pair `pr` together: the two score
            matmuls for one s-tile live in disjoint PE row-groups (lhsT rows
            0-63 vs 64-127) and run concurrently on hardware; one [128,1024]
            ACT exp covers both heads.  Yields once per s-tile so the driver
            can weave filler PE work into the exp-latency bubbles."""
            t0 = qb * TBLK
            ns = 4 * (qb + 1)
            attv = [
                psB.tile([VW, TBLK], F32, tag="attv", name=f"attv{qb}_{pr}_{par}")
                for par in range(2)
            ]
            for si in range(ns):
                diag = si * P >= t0
                ka = si * P - t0 if diag else 0
                stp = ps2.tile([P, 2 * TBLK], F32, tag="st2",
                               name=f"st{qb}_{pr}_{si}")
                for par in range(2):
                    r0 = par * HS
                    nc.tensor.matmul(
                        stp[:, par * TBLK:(par + 1) * TBLK],
                        lhsT=kt_slice(pr, r0, HS, si * P, P),
                        rhs=qt_slice(pr, r0, HS, t0, TBLK),
                        start=True, stop=True,
                    )
                pt = ptp.tile([P, 2 * TBLK], F32R, tag="pt",
                              name=f"pt{qb}_{pr}_{si}")
                if diag:
                    for par in range(2):
                        c0 = par * TBLK + ka
                        nc.scalar.activation(
                            pt[:, c0:(par + 1) * TBLK],
                            stp[:, c0:(par + 1) * TBLK], EXP, scale=0.125)
                        nc.vector.tensor_mul(
                            pt[:, c0:c0 + P], pt[:, c0:c0 + P], tri_sb[:])
                else:
                    nc.scalar.activation(pt[:], stp[:], EXP, scale=0.125)
                for par in range(2):
                    h = 2 * pr + par
                    nc.tensor.matmul(
                        attv[par][:, ka:TBLK],
                        lhsT=v_sb[si][:, h * VW:(h + 1) * VW],
                        rhs=pt[:, par * TBLK + ka:(par + 1) * TBLK],
                        start=(si == 0), stop=(si == ns - 1),
                    )
                yield
            # normalize: recip of denominator row, PE-broadcast, multiply
            for par in range(2):
                r0 = par * HS
                recip = smp.tile([1, TBLK], F32R, tag="recip")
                with nc.allow_low_precision(
                    reason="f32r reciprocal feeds PE broadcast; 19-bit "
                    "mantissa is ample for softmax denominators"
                ):
                    nc.vector.reciprocal(recip[:], attv[par][HS:HS + 1, :])
                bc = psC.tile([HS, TBLK], F32, tag="small",
                              name=f"bc{qb}_{pr}_{par}")
                nc.tensor.matmul(
                    bc[:], lhsT=ones[0:1, 0:HS], rhs=recip[:],
                    start=True, stop=True,
                )
                bcs = smp.tile([HS, TBLK], F32, tag="bcs")
                nc.vector.tensor_copy(bcs[:], bc[:])
                nc.vector.tensor_mul(
                    attn_pair[pr][r0:r0 + HS, :], attv[par][0:HS, :], bcs[:]
                )

        # ---------------- stage-3 piece (one t-tile of one t-block) ---------
        def emit_oproj_tt(qb, lh, tt):
            # lh[hdt] holds att^T rows for global heads (2*hdt, 2*hdt+1)...
            # here indexed so lh[hdt] pairs with w_sb["wo"][hdt]
            t0 = qb * TBLK
            op = psC.tile([P, ES], F32, tag="small", name=f"op{qb}_{tt}")
            nc.tensor.matmul(
                op[:], lhsT=ones[0:1, :], rhs=bias_sb[:],
                start=True, stop=False,
            )
            # pr0 tiles (even hdt) first: they arrive one AllGather earlier
            order = [0, 2, 4, 6, 1, 3, 5, 7]
            for i, hdt in enumerate(order):
                nc.tensor.matmul(
                    op[:],
                    lhsT=lh[hdt][:, tt * P:(tt + 1) * P],
                    rhs=w_sb["wo"][hdt][:],
                    start=False,
                    stop=(i == NCT - 1),
                )
            osb = outp.tile([P, ES], F32, tag="osb", name=f"osb{qb}_{tt}")
            nc.scalar.copy(osb[:], op[:])
            nc.sync.dma_start(out[t0 + tt * P:t0 + (tt + 1) * P, :], osb[:])

        # --------- per-pair AllGather (pr = head pair 0/1 of this core) -----
        # Gathering one head-pair [128, 512] per collective: output rows are
        # rank-major, i.e. block g holds GLOBAL heads (4g+2pr, 4g+2pr+1) =
        # global hd-tile index 2g+pr.  lh list is indexed by wo-row tile.
        def emit_ag(qb, pr, attn_pair, lh):
            ag_in = dramp.tile([P, TBLK], F32R, tag=f"agin{qb}_{pr}")
            nc.sync.dma_start(ag_in[:], attn_pair[pr][:])
            ag_out = dramp.tile([GROUPS * P, TBLK], F32R, tag=f"agout{qb}_{pr}")
            if with_collective:
                nc.gpsimd.collective_compute(
                    "AllGather",
                    mybir.AluOpType.bypass,
                    replica_groups=REPLICA_GROUPS,
                    ins=[ag_in[:].opt()],
                    outs=[ag_out[:].opt()],
                )
            else:  # timing/sim variant: fake the AG with local DMA copies
                for g_ in range(GROUPS):
                    nc.sync.dma_start(
                        ag_out[g_ * P:(g_ + 1) * P, :], ag_in[:])
            for g_ in range(GROUPS):
                t_ = lhp.tile([P, TBLK], F32R, tag="lh",
                              name=f"lh{qb}_{pr}_{g_}")
                eng = nc.sync if g_ % 2 == 0 else nc.gpsimd
                eng.dma_start(t_[:], ag_out[g_ * P:(g_ + 1) * P, :])
                lh[2 * g_ + pr] = t_

        # ---------------- emission schedule ----------------
        # stage 1, t-block 0 (DMAs interleaved for fast start)
        xt0 = alloc_xt(0)
        for ci in range(NCT):
            nc.sync.dma_start(w_sb["wq"][ci][:], wq[ci * P:(ci + 1) * P, :])
            nc.gpsimd.dma_start(xt0[ci][:], xT[ci * P:(ci + 1) * P, 0:TBLK])
        for name, src in (("wk", wk), ("wv", wv)):
            for ci in range(NCT):
                nc.sync.dma_start(w_sb[name][ci][:], src[ci * P:(ci + 1) * P, :])
        for chunk in qk_chunks(0) + v_chunks(0):
            chunk()

        lh_of = {}
        for qb in range(NTB):
            if qb + 1 < NTB:
                emit_x_dma(qb + 1)
            if qb == 0:
                # wo/bias DMAs: needed only from stage 3 on, so they queue
                # behind the t-block-1 x loads
                for ci in range(NCT):
                    nc.sync.dma_start(
                        w_sb["wo"][ci][:], wo[ci * P:(ci + 1) * P, :])
                nc.sync.dma_start(bias_sb[:], bo[:])
            # V projections of t-block qb (deferred from stage 1): consumed
            # by this block's own diagonal s-tiles, so they are injected at
            # units 2/4/6/8 of the first head pair.  Other fillers (next
            # block's Q/K projections, previous block's out-projection) are
            # spread evenly.
            vfill = v_chunks(qb) if qb > 0 else []
            fillers = qk_chunks(qb + 1) if qb + 1 < NTB else []
            if qb > 0:
                fillers += [
                    (lambda tt=tt, q=qb - 1: emit_oproj_tt(q, lh_of[q], tt))
                    for tt in range(4)
                ]
            total_units = 2 * 4 * (qb + 1)
            last = qb == NTB - 1
            # on the last t-block, keep half the fillers for after the final
            # AllGather is issued: they are the only PE work that can cover
            # the normalize + collective latency of the tail
            navail = max(1, len(fillers) // 2) if last else max(1, len(fillers))
            stride = max(2, total_units // navail)
            attn_pair = [
                attp.tile([P, TBLK], F32R, tag=f"attn{p_}", name=f"at{qb}_{p_}")
                for p_ in range(2)
            ]
            lh = [None] * NCT
            ctr = 0
            for pr in range(2):
                for _ in emit_headpair(qb, pr, attn_pair):
                    ctr += 1
                    if vfill and ctr % 2 == 1:
                        vfill.pop(0)()
                    elif fillers and ctr % stride == 0:
                        fillers.pop(0)()
                if pr == 0:
                    emit_ag(qb, 0, attn_pair, lh)
            while vfill:
                vfill.pop(0)()
            if not last:
                while fillers:
                    fillers.pop(0)()
            emit_ag(qb, 1, attn_pair, lh)
            while fillers:
                fillers.pop(0)()
            lh_of[qb] = lh

        # tail: out-projection of the last t-block
        for tt in range(4):
            emit_oproj_tt(NTB - 1, lh_of[NTB - 1], tt)

    nc.compile()
    return nc


_NC_CACHE = {}


def _get_nc(with_collective=True):
    key = with_collective
    if key not in _NC_CACHE:
        _NC_CACHE[key] = build_nc(with_collective)
    return _NC_CACHE[key]


def make_in_maps(x, Wq, Wk, Wv, Wo, bo):
    tri = np.ascontiguousarray(np.triu(np.ones((P, P), dtype=np.float32)))
    onesc = np.ones((1, P), dtype=np.float32)
    vones = np.ones((P, HPG), dtype=np.float32)
    in_maps = []
    for c in range(N_CORES):
        b, g = c // GROUPS, c % GROUPS
        hs_ = slice(g * HPG, (g + 1) * HPG)
        in_maps.append({
            "xT": np.ascontiguousarray(x[b].T),
            "wq": np.ascontiguousarray(
                Wq[hs_].transpose(1, 0, 2).reshape(C, HD)),
            "wk": np.ascontiguousarray(
                Wk[hs_].transpose(1, 0, 2).reshape(C, HD)),
            "wv": np.ascontiguousarray(
                Wv[hs_].transpose(1, 0, 2).reshape(C, HD)),
            "wo": np.ascontiguousarray(Wo[:, g * ES:(g + 1) * ES]),
            "bo": np.ascontiguousarray(bo[g * ES:(g + 1) * ES].reshape(1, ES)),
            "tri": tri,
            "onesc": onesc,
            "vones": vones,
        })
    return in_maps


def kernel(x, Wq, Wk, Wv, Wo, bo):
    x = np.asarray(x, dtype=np.float32)
    Wq = np.asarray(Wq, dtype=np.float32)
    Wk = np.asarray(Wk, dtype=np.float32)
    Wv = np.asarray(Wv, dtype=np.float32)
    Wo = np.asarray(Wo, dtype=np.float32)
    bo = np.asarray(bo, dtype=np.float32)

    nc = _get_nc(with_collective=True)
    in_maps = make_in_maps(x, Wq, Wk, Wv, Wo, bo)
    res = run_bass_kernel_spmd(nc, in_maps, core_ids=list(range(N_CORES)))

    out = np.empty((B, T, E), dtype=np.float32)
    for c in range(N_CORES):
        b, g = c // GROUPS, c % GROUPS
        out[b, :, g * ES:(g + 1) * ES] = res.results[c]["out"]
    return out


# revision 44
# speedup vs baseline: 1.0824x; 1.0024x over previous
"""Multi-head causal self-attention on 8 TRN2 NeuronCores.

Problem (nn_MultiHeadAttention): B=2, T=2048, C=1024, H=16 heads, hs=64.
  q,k,v = per-head projections of x; causal softmax(q k^T / 8) v;
  concat heads; out = att @ Wo + bo.

Sharding: core c in 0..7 -> (batch b = c//4, head-group g = c%4, 4 heads each).
Each core computes Q/K/V + flash-style causal attention for its 4 heads on its
batch, normalized attention outputs are AllGathered across the 4 cores of the
same batch (replica groups [0-3], [4-7]), then each core computes a disjoint
256-column slice of the output projection (column-parallel Wo) + bias slice.
Host does a pure concat of the 8 disjoint output slices.

All matmuls run as float32r (single-pass fp32 PE mode, 4x faster than fp32).
Attention works in transposed layout throughout: Q^T/K^T [d, t], scores
S^T [s, t], P^T = exp(S^T/8) with causal mask, att^T [d, t] via
lhsT=[V_h | ones] (row 64 of the PSUM accumulator = softmax denominator).
Normalization multiplies by a PE-broadcast reciprocal row.

Scheduling notes: per-engine instruction order is static, so projection
(stage 1) and output-projection (stage 3) work is interleaved into the
attention head loops to fill PE bubbles left by the scores->exp->AV chain,
and stage-3 matmuls for t-block qb are emitted only during stage-2 of qb+1,
when their AllGathered inputs have already landed.
"""

import numpy as np
from contextlib import ExitStack

import concourse.bass as bass
import concourse.mybir as mybir
import concourse.tile as tile
from concourse import bacc
from concourse.bass_utils import run_bass_kernel_spmd

F32 = mybir.dt.float32
F32R = mybir.dt.float32r
EXP = mybir.ActivationFunctionType.Exp

N_CORES = 8
B = 2
T = 2048
C = 1024
NH = 16
HS = 64
E = 1024
GROUPS = 4          # head groups (tensor-parallel ranks per batch)
HPG = NH // GROUPS  # 4 heads per core
ES = E // GROUPS    # 256 output columns per core
HD = HPG * HS       # 256 local attention-output rows

P = 128             # partition tile
TBLK = 512          # t-block (matmul moving dim)
NTB = T // TBLK     # 4
NCT = C // P        # 8 contraction tiles for projections
NST = T // P        # 16 key tiles
VW = HS + 1         # V lhsT width per head (64 V cols + ones col)

REPLICA_GROUPS = [[0, 1, 2, 3], [4, 5, 6, 7]]


def build_nc(with_collective=True):
    """Build + compile the per-core SPMD program. Same program on all cores."""
    nc = bacc.Bacc(
        "TRN2", target_bir_lowering=False, debug=False, num_devices=N_CORES
    )

    xT = nc.dram_tensor("xT", [C, T], F32R, kind="ExternalInput").ap()
    wq = nc.dram_tensor("wq", [C, HD], F32R, kind="ExternalInput").ap()
    wk = nc.dram_tensor("wk", [C, HD], F32R, kind="ExternalInput").ap()
    wv = nc.dram_tensor("wv", [C, HD], F32R, kind="ExternalInput").ap()
    wo = nc.dram_tensor("wo", [E, ES], F32R, kind="ExternalInput").ap()
    bo = nc.dram_tensor("bo", [1, ES], F32R, kind="ExternalInput").ap()
    tri = nc.dram_tensor("tri", [P, P], F32R, kind="ExternalInput").ap()
    onesc = nc.dram_tensor("onesc", [1, P], F32R, kind="ExternalInput").ap()
    vones = nc.dram_tensor("vones", [P, HPG], F32R, kind="ExternalInput").ap()
    out = nc.dram_tensor("out", [T, ES], F32, kind="ExternalOutput").ap()

    with tile.TileContext(nc) as tc, ExitStack() as ctx:
        wp = ctx.enter_context(tc.tile_pool(name="wp", bufs=1))
        xp = ctx.enter_context(tc.tile_pool(name="xp", bufs=2))
        qkp = ctx.enter_context(tc.tile_pool(name="qkp", bufs=1))
        vp = ctx.enter_context(tc.tile_pool(name="vp", bufs=1))
        ptp = ctx.enter_context(tc.tile_pool(name="ptp", bufs=8))
        attp = ctx.enter_context(tc.tile_pool(name="attp", bufs=2))
        smp = ctx.enter_context(tc.tile_pool(name="smp", bufs=4))
        outp = ctx.enter_context(tc.tile_pool(name="outp", bufs=3))
        lhp = ctx.enter_context(tc.tile_pool(name="lhp", bufs=10))
        # PSUM: 8 banks total.  st2 [128,1024] = 2 banks x 2 bufs = 4,
        # attv 1 bank x 2, small (bc / out-proj) 1 bank x 2.
        ps2 = ctx.enter_context(tc.tile_pool(name="ps2", bufs=2, space="PSUM"))
        psB = ctx.enter_context(tc.tile_pool(name="psB", bufs=2, space="PSUM"))
        psC = ctx.enter_context(tc.tile_pool(name="psC", bufs=2, space="PSUM"))
        dramp = ctx.enter_context(tc.tile_pool(name="dramp", bufs=1, space="DRAM"))

        # ---- small constants ----
        ones = wp.tile([1, P], F32R, tag="ones")
        nc.sync.dma_start(ones[:], onesc[:])
        tri_sb = wp.tile([P, P], F32R, tag="tri")
        nc.sync.dma_start(tri_sb[:], tri[:])
        bias_sb = wp.tile([1, ES], F32R, tag="bias")

        w_sb = {n: [] for n in ("wq", "wk", "wv", "wo")}
        for name in ("wq", "wk", "wv", "wo"):
            for ci in range(NCT):
                w_sb[name].append(
                    wp.tile([P, ES], F32R, tag=f"{name}{ci}", name=f"{name}{ci}")
                )

        # x^T tiles per (c-tile, t-block), double-buffered across t-blocks:
        # x(tb) is only read by stage-1(tb), so two t-blocks' worth suffices
        xt_of = {}

        def alloc_xt(tb):
            xt_of[tb] = [
                xp.tile([P, TBLK], F32R, tag=f"x{ci}", name=f"x{ci}_{tb}")
                for ci in range(NCT)
            ]
            return xt_of[tb]

        # merged Q^T/K^T per head pair: col = tb*1024 + qk*512 + t_local
        # (pair p holds heads 2p (rows 0-63) and 2p+1 (rows 64-127))
        qkt = [qkp.tile([P, 2 * T], F32R, tag=f"qk{p_}", name=f"qk{p_}")
               for p_ in range(2)]

        def qt_slice(pr, r0, rn, t0, tn):
            tb, tl = t0 // TBLK, t0 % TBLK
            base = tb * 1024 + tl
            return qkt[pr][r0:r0 + rn, base:base + tn]

        def kt_slice(pr, r0, rn, s0, sn):
            tb, sl = s0 // TBLK, s0 % TBLK
            base = tb * 1024 + TBLK + sl
            return qkt[pr][r0:r0 + rn, base:base + sn]

        v_sb = [vp.tile([P, HPG * VW], F32R, tag=f"v{st}", name=f"v{st}")
                for st in range(NST)]

        # ---------------- stage-1 pieces ----------------
        def emit_x_dma(tb):
            # SWDGE queues (gpsimd): runs parallel to the HWDGE weight loads
            ts_ = tb * TBLK
            xt = alloc_xt(tb)
            for ci in range(NCT):
                nc.gpsimd.dma_start(
                    xt[ci][:], xT[ci * P:(ci + 1) * P, ts_:ts_ + TBLK])

        def emit_qk_proj(tb, pr, which):
            # one [128,512] accumulation on the psC "small" tag (see
            # emit_v_proj for why not st2); which=0 -> Q, which=1 -> K
            xt = xt_of[tb]
            wn = "wq" if which == 0 else "wk"
            ps = psC.tile([P, TBLK], F32, tag="small",
                          name=f"qkps{tb}_{pr}_{which}")
            for ci in range(NCT):
                nc.tensor.matmul(
                    ps[:],
                    lhsT=w_sb[wn][ci][:, pr * P:(pr + 1) * P],
                    rhs=xt[ci][:],
                    start=(ci == 0), stop=(ci == NCT - 1),
                )
            base = tb * 1024 + which * TBLK
            nc.vector.tensor_copy(qkt[pr][:, base:base + TBLK], ps[:])

        def emit_v_proj(st):
            # psC "small" tag, NOT ps2: a V filler holding an st2 slot for its
            # 8-matmul group would degrade the scores/exp pipeline to
            # single-buffering
            tb, sl = st // 4, (st % 4) * P
            xt = xt_of[tb]
            vps = psC.tile([P, TBLK], F32, tag="small", name=f"vps{st}")
            for ci in range(NCT):
                nc.tensor.matmul(
                    vps[:, 0:HD],
                    lhsT=xt[ci][:, sl:sl + P],
                    rhs=w_sb["wv"][ci][:],
                    start=(ci == 0), stop=(ci == NCT - 1),
                )
            nc.sync.dma_start(
                v_sb[st][:].rearrange("p (h x) -> p h x", h=HPG)[:, :, HS:VW],
                vones[:].rearrange("p (h o) -> p h o", o=1),
            )
            nc.vector.tensor_copy(
                v_sb[st][:].rearrange("p (h x) -> p h x", h=HPG)[:, :, 0:HS],
                vps[:, 0:HD].rearrange("p (h x) -> p h x", h=HPG),
            )

        def qk_chunks(tb):
            return [lambda tb=tb, pr=pr, w=w: emit_qk_proj(tb, pr, w)
                    for pr in range(2) for w in range(2)]

        def v_chunks(tb):
            return [lambda st=st: emit_v_proj(st)
                    for st in range(4 * tb, 4 * tb + 4)]

        # ------- stage-2 piece (one head PAIR of one t-block, jointly) ------
        def emit_headpair(qb, pr, attn_pair):
            """Process both heads of qkt pair `pr` together: the two score
            matmuls for one s-tile live in disjoint PE row-groups (lhsT rows
            0-63 vs 64-127) and run concurrently on hardware; one [128,1024]
            ACT exp covers both heads.  Yields once per s-tile so the driver
            can weave filler PE work into the exp-latency bubbles."""
            t0 = qb * TBLK
            ns = 4 * (qb + 1)
            attv = [
                psB.tile([VW, TBLK], F32, tag="attv", name=f"attv{qb}_{pr}_{par}")
                for par in range(2)
            ]
            for si in range(ns):
                diag = si * P >= t0
                ka = si * P - t0 if diag else 0
                stp = ps2.tile([P, 2 * TBLK], F32, tag="st2",
                               name=f"st{qb}_{pr}_{si}")
                for par in range(2):
                    r0 = par * HS
                    nc.tensor.matmul(
                        stp[:, par * TBLK:(par + 1) * TBLK],
                        lhsT=kt_slice(pr, r0, HS, si * P, P),
                        rhs=qt_slice(pr, r0, HS, t0, TBLK),
                        start=True, stop=True,
                    )
                pt = ptp.tile([P, 2 * TBLK], F32R, tag="pt",
                              name=f"pt{qb}_{pr}_{si}")
                if diag:
                    for par in range(2):
                        c0 = par * TBLK + ka
                        nc.scalar.activation(
                            pt[:, c0:(par + 1) * TBLK],
                            stp[:, c0:(par + 1) * TBLK], EXP, scale=0.125)
                        nc.vector.tensor_mul(
                            pt[:, c0:c0 + P], pt[:, c0:c0 + P], tri_sb[:])
                else:
                    nc.scalar.activation(pt[:], stp[:], EXP, scale=0.125)
                for par in range(2):
                    h = 2 * pr + par
                    nc.tensor.matmul(
                        attv[par][:, ka:TBLK],
                        lhsT=v_sb[si][:, h * VW:(h + 1) * VW],
                        rhs=pt[:, par * TBLK + ka:(par + 1) * TBLK],
                        start=(si == 0), stop=(si == ns - 1),
                    )
                yield
            # normalize: recip of denominator row, PE-broadcast, multiply
            for par in range(2):
                r0 = par * HS
                recip = smp.tile([1, TBLK], F32R, tag="recip")
                with nc.allow_low_precision(
                    reason="f32r reciprocal feeds PE broadcast; 19-bit "
                    "mantissa is ample for softmax denominators"
                ):
                    nc.vector.reciprocal(recip[:], attv[par][HS:HS + 1, :])
                bc = psC.tile([HS, TBLK], F32, tag="small",
                              name=f"bc{qb}_{pr}_{par}")
                nc.tensor.matmul(
                    bc[:], lhsT=ones[0:1, 0:HS], rhs=recip[:],
                    start=True, stop=True,
                )
                bcs = smp.tile([HS, TBLK], F32, tag="bcs")
                nc.vector.tensor_copy(bcs[:], bc[:])
                nc.vector.tensor_mul(
                    attn_pair[pr][r0:r0 + HS, :], attv[par][0:HS, :], bcs[:]
                )

        # ---------------- stage-3 piece (one t-tile of one t-block) ---------
        def emit_oproj_tt(qb, lh, tt):
            # lh[hdt] holds att^T rows for global heads (2*hdt, 2*hdt+1)...
            # here indexed so lh[hdt] pairs with w_sb["wo"][hdt]
            t0 = qb * TBLK
            op = psC.tile([P, ES], F32, tag="small", name=f"op{qb}_{tt}")
            nc.tensor.matmul(
                op[:], lhsT=ones[0:1, :], rhs=bias_sb[:],
                start=True, stop=False,
            )
            # pr0 tiles (even hdt) first: they arrive one AllGather earlier
            order = [0, 2, 4, 6, 1, 3, 5, 7]
            for i, hdt in enumerate(order):
                nc.tensor.matmul(
                    op[:],
                    lhsT=lh[hdt][:, tt * P:(tt + 1) * P],
                    rhs=w_sb["wo"][hdt][:],
                    start=False,
                    stop=(i == NCT - 1),
                )
            osb = outp.tile([P, ES], F32, tag="osb", name=f"osb{qb}_{tt}")
            nc.scalar.copy(osb[:], op[:])
            nc.sync.dma_start(out[t0 + tt * P:t0 + (tt + 1) * P, :], osb[:])

        # --------- per-pair AllGather (pr = head pair 0/1 of this core) -----
        # Gathering one head-pair [128, 512] per collective: output rows are
        # rank-major, i.e. block g holds GLOBAL heads (4g+2pr, 4g+2pr+1) =
        # global hd-tile index 2g+pr.  lh list is indexed by wo-row tile.
        def emit_ag(qb, pr, attn_pair, lh):
            ag_in = dramp.tile([P, TBLK], F32R, tag=f"agin{qb}_{pr}")
            nc.sync.dma_start(ag_in[:], attn_pair[pr][:])
            ag_out = dramp.tile([GROUPS * P, TBLK], F32R, tag=f"agout{qb}_{pr}")
            if with_collective:
                nc.gpsimd.collective_compute(
                    "AllGather",
                    mybir.AluOpType.bypass,
                    replica_groups=REPLICA_GROUPS,
                    ins=[ag_in[:].opt()],
                    outs=[ag_out[:].opt()],
                )
            else:  # timing/sim variant: fake the AG with local DMA copies
                for g_ in range(GROUPS):
                    nc.sync.dma_start(
                        ag_out[g_ * P:(g_ + 1) * P, :], ag_in[:])
            for g_ in range(GROUPS):
                t_ = lhp.tile([P, TBLK], F32R, tag="lh",
                              name=f"lh{qb}_{pr}_{g_}")
                eng = nc.sync if g_ % 2 == 0 else nc.gpsimd
                eng.dma_start(t_[:], ag_out[g_ * P:(g_ + 1) * P, :])
                lh[2 * g_ + pr] = t_

        # ---------------- emission schedule ----------------
        # stage 1, t-block 0 (DMAs interleaved for fast start)
        xt0 = alloc_xt(0)
        for ci in range(NCT):
            nc.sync.dma_start(w_sb["wq"][ci][:], wq[ci * P:(ci + 1) * P, :])
            nc.gpsimd.dma_start(xt0[ci][:], xT[ci * P:(ci + 1) * P, 0:TBLK])
        for name, src in (("wk", wk), ("wv", wv)):
            for ci in range(NCT):
                nc.sync.dma_start(w_sb[name][ci][:], src[ci * P:(ci + 1) * P, :])
        for chunk in qk_chunks(0) + v_chunks(0):
            chunk()

        lh_of = {}
        for qb in range(NTB):
            if qb + 1 < NTB:
                emit_x_dma(qb + 1)
            if qb == 0:
                # wo/bias DMAs: needed only from stage 3 on, so they queue
                # behind the t-block-1 x loads
                for ci in range(NCT):
                    nc.sync.dma_start(
                        w_sb["wo"][ci][:], wo[ci * P:(ci + 1) * P, :])
                nc.sync.dma_start(bias_sb[:], bo[:])
            # V projections of t-block qb (deferred from stage 1): consumed
            # by this block's own diagonal s-tiles, so they are injected at
            # units 2/4/6/8 of the first head pair.  Other fillers (next
            # block's Q/K projections, previous block's out-projection) are
            # spread evenly.
            vfill = v_chunks(qb) if qb > 0 else []
            fillers = qk_chunks(qb + 1) if qb + 1 < NTB else []
            if qb > 0:
                fillers += [
                    (lambda tt=tt, q=qb - 1: emit_oproj_tt(q, lh_of[q], tt))
                    for tt in range(4)
                ]
            total_units = 2 * 4 * (qb + 1)
            last = qb == NTB - 1
            # on the last t-block, keep half the fillers for after the final
            # AllGather is issued: they are the only PE work that can cover
            # the normalize + collective latency of the tail
            navail = max(1, len(fillers) // 2) if last else max(1, len(fillers))
            stride = max(2, total_units // navail)
            attn_pair = [
                attp.tile([P, TBLK], F32R, tag=f"attn{p_}", name=f"at{qb}_{p_}")
                for p_ in range(2)
            ]
            lh = [None] * NCT
            ctr = 0
            for pr in range(2):
                for _ in emit_headpair(qb, pr, attn_pair):
                    ctr += 1
                    if vfill and ctr % 2 == 1:
                        vfill.pop(0)()
                    elif fillers and ctr % stride == 0:
                        fillers.pop(0)()
                if pr == 0:
                    emit_ag(qb, 0, attn_pair, lh)
            while vfill:
                vfill.pop(0)()
            if not last:
                while fillers:
                    fillers.pop(0)()
            emit_ag(qb, 1, attn_pair, lh)
            while fillers:
                fillers.pop(0)()
            lh_of[qb] = lh

        # tail: out-projection of the last t-block
        for tt in range(4):
            emit_oproj_tt(NTB - 1, lh_of[NTB - 1], tt)

    nc.compile()
    return nc


_NC_CACHE = {}


def _get_nc(with_collective=True):
    key = with_collective
    if key not in _NC_CACHE:
        _NC_CACHE[key] = build_nc(with_collective)
    return _NC_CACHE[key]


def make_in_maps(x, Wq, Wk, Wv, Wo, bo):
    tri = np.ascontiguousarray(np.triu(np.ones((P, P), dtype=np.float32)))
    onesc = np.ones((1, P), dtype=np.float32)
    vones = np.ones((P, HPG), dtype=np.float32)
    in_maps = []
    for c in range(N_CORES):
        b, g = c // GROUPS, c % GROUPS
        hs_ = slice(g * HPG, (g + 1) * HPG)
        in_maps.append({
            "xT": np.ascontiguousarray(x[b].T),
            "wq": np.ascontiguousarray(
                Wq[hs_].transpose(1, 0, 2).reshape(C, HD)),
            "wk": np.ascontiguousarray(
                Wk[hs_].transpose(1, 0, 2).reshape(C, HD)),
            "wv": np.ascontiguousarray(
                Wv[hs_].transpose(1, 0, 2).reshape(C, HD)),
            "wo": np.ascontiguousarray(Wo[:, g * ES:(g + 1) * ES]),
            "bo": np.ascontiguousarray(bo[g * ES:(g + 1) * ES].reshape(1, ES)),
            "tri": tri,
            "onesc": onesc,
            "vones": vones,
        })
    return in_maps


def kernel(x, Wq, Wk, Wv, Wo, bo):
    x = np.asarray(x, dtype=np.float32)
    Wq = np.asarray(Wq, dtype=np.float32)
    Wk = np.asarray(Wk, dtype=np.float32)
    Wv = np.asarray(Wv, dtype=np.float32)
    Wo = np.asarray(Wo, dtype=np.float32)
    bo = np.asarray(bo, dtype=np.float32)

    nc = _get_nc(with_collective=True)
    in_maps = make_in_maps(x, Wq, Wk, Wv, Wo, bo)
    res = run_bass_kernel_spmd(nc, in_maps, core_ids=list(range(N_CORES)))

    out = np.empty((B, T, E), dtype=np.float32)
    for c in range(N_CORES):
        b, g = c // GROUPS, c % GROUPS
        out[b, :, g * ES:(g + 1) * ES] = res.results[c]["out"]
    return out
